# revision 16
# baseline (speedup 1.0000x reference)
"""DCT band-decomposition kernel for Trainium2 (8 NeuronCores, SPMD).

Math: for each 8x8 block X of the input image, the reference computes
C = D @ X @ D^T, then per band b: out_b = D^T @ (M_b * C) @ D * scale_b.
This is linear in the 64 block elements, so each band is one fixed 64x64
matrix A_b = (D^T (x) D^T) diag(vec(M_b)) (D (x) D) applied to the
block-vectorized input.  Masks partition the spectrum, so
A_low + A_mid + A_high = I  =>  high_u = x - low_u - mid_u (done on host).

Device work per core (batch-sharded, 2 images of [3,512,512] per core):
  - input xv [128, 12288] f32: partitions 0-63 hold even blocks
    (vectorized 8x8), partitions 64-127 odd blocks; free = block pairs.
  - Two row-tiled matmuls per 512-column chunk run concurrently in
    disjoint PE row strips (K=64 each, auto tile_position from base
    partition 0 / 64).  lhsT = [A_low^T | A_mid^T] (M=128 = both bands).
  - PSUM -> DVE/ACT copies -> SBUF staging -> 2 MiB HWDGE DMA out.
  - Input streamed in 1 MiB SWDGE chunks so in/out DMA interleave.
Host folds band_scale (ones in practice) and restores natural layout.
"""

import numpy as np

N_CORES = 8
B, C, H, W = 16, 3, 512, 512
NB = 8  # DCT block size
B_LOC = B // N_CORES  # 2 images per core
NBH, NBW = H // NB, W // NB  # 64 x 64 blocks
FREE = B_LOC * C * NBH * (NBW // 2)  # 12288 block-pair columns per core
IN_CHUNK = 2048  # input DMA chunk (1 MiB)
GROUP = 2048  # columns per output DMA (1 MiB)
NG = FREE // GROUP  # 6
PS_COLS = 1024  # PSUM tile = 2 banks
MM_N = 512  # matmul free dim (one PSUM bank of f32)
DVE_COLS = 640  # per-PSUM-tile copy split: DVE 640 cols, ACT 384


def _dct_matrix(n):
    m = np.zeros((n, n), dtype=np.float64)
    for k in range(n):
        for t in range(n):
            if k == 0:
                m[k, t] = np.sqrt(1.0 / n)
            else:
                m[k, t] = np.sqrt(2.0 / n) * np.cos(np.pi * k * (2 * t + 1) / (2 * n))
    return m


def _zigzag(n):
    idxs = np.zeros((n, n), dtype=np.int64)
    idx = 0
    for s in range(2 * n - 1):
        if s % 2 == 0:
            rng = range(min(s, n - 1), max(0, s - n + 1) - 1, -1)
        else:
            rng = range(max(0, s - n + 1), min(s, n - 1) + 1)
        for i in rng:
            j = s - i
            if 0 <= i < n and 0 <= j < n:
                idxs[i, j] = idx
                idx += 1
    return idxs


def _band_weights():
    """lhsT [128, 128]: rows 0-63 / 64-127 both hold [A_low^T | A_mid^T]."""
    D = _dct_matrix(NB)
    zz = _zigzag(NB)
    total = NB * NB
    lo_t, hi_t = total // 3, 2 * total // 3
    low = (zz < lo_t).astype(np.float64)
    mid = ((zz >= lo_t) & (zz < hi_t)).astype(np.float64)
    K64 = np.kron(D, D)  # vec_row(D X D^T) = K64 @ vec_row(X)
    blocks = []
    for mask in (low, mid):
        A = K64.T @ (mask.reshape(-1)[:, None] * K64)  # 64x64 band operator
        blocks.append(A.T)  # out = lhsT.T @ rhs
    wA = np.concatenate(blocks, axis=1)  # [64, 128]
    w = np.concatenate([wA, wA], axis=0)  # [128, 128]
    return np.ascontiguousarray(w.astype(np.float32))


def _pack(xc):
    """[2,3,512,512] -> [128, 12288] block-pair-vectorized layout."""
    # (b, ch, bh, r, bw2, par, c) <- H = bh*8 + r ; W = bw2*16 + par*8 + c
    v = xc.reshape(B_LOC, C, NBH, NB, NBW // 2, 2, NB)
    v = v.transpose(5, 3, 6, 0, 1, 2, 4)  # (par, r, c, b, ch, bh, bw2)
    return np.ascontiguousarray(v.reshape(128, FREE))


def _unpack(yv):
    """[128, 12288] -> [2,3,512,512] (inverse of _pack)."""
    v = yv.reshape(2, NB, NB, B_LOC, C, NBH, NBW // 2)
    v = v.transpose(3, 4, 5, 1, 6, 0, 2)  # (b, ch, bh, r, bw2, par, c)
    return v.reshape(B_LOC, C, H, W)


IN_CHUNK = 4096  # 2 MiB input chunks
GROUP = 4096  # output DMA cols per stream (2 MiB)
NG = FREE // GROUP  # 3
PS_COLS = 1024
HPG = GROUP // PS_COLS  # 4 psum subtiles per output group
MM_N = 512
DVE_COLS = 640
N_PS_SLOTS = 4  # [128,1024] = 2 banks each -> all 8 PSUM banks
N_ST_SLOTS = 4  # stage ring [128, 2048] each

_CACHE = {}


def _build_raw():
    if "nc" in _CACHE:
        return _CACHE["nc"]
    from concourse import bacc, mybir

    f32 = mybir.dt.float32
    nc = bacc.Bacc("TRN2", target_bir_lowering=False, debug=False, num_devices=N_CORES)
    xv = nc.dram_tensor("xv", [128, FREE], f32, kind="ExternalInput").ap()
    w = nc.dram_tensor("w", [128, 128], f32, kind="ExternalInput").ap()
    yv = nc.dram_tensor("yv", [2, 128, FREE], f32, kind="ExternalOutput").ap()

    xbuf = nc.alloc_sbuf_tensor("xbuf", [128, FREE], f32).ap()
    wt = nc.alloc_sbuf_tensor("wt", [128, 128], f32).ap()
    st = nc.alloc_sbuf_tensor("st", [128, N_ST_SLOTS * GROUP], f32).ap()
    ps = nc.alloc_psum_tensor("ps", [128, N_PS_SLOTS * PS_COLS], f32).ap()

    n_in = FREE // IN_CHUNK  # 6
    n_out = NG * 2  # 12 output DMAs: (g, s)

    def tile_idx(g, h, s):
        return (g * HPG + h) * 2 + s

    with (
        # GpSimd issues no DMAs in this kernel, so skip its dge_drain in
        # the block-exit barrier (sem-only barrier instead).
        nc.Block(no_gpsimd_drain=True) as block,
        nc.semaphore("sem_in") as sem_in,
        nc.semaphore("sem_mm") as sem_mm,
        nc.semaphore("sem_cpv") as sem_cpv,
        nc.semaphore("sem_cps") as sem_cps,
        nc.semaphore("sem_out") as sem_out,
    ):
        # Inputs ride the ACT HWDGE ring (issued at the head of the scalar
        # program), outputs ride the SP ring, so output bytes can start
        # moving while input chunks are still streaming.
        @block.sync
        def _(sync):
            n_dma = 0
            for g in range(NG):
                for s in range(2):
                    m = 2 * g + s
                    slot = m % N_ST_SLOTS
                    # Split the final group's transfers in half: the last
                    # DMA is a serial tail after the last copies, so the
                    # first half can overlap the second half's copies.
                    halves = 2 if g == NG - 1 else 1
                    hw = GROUP // halves
                    for part in range(halves):
                        k = tile_idx(g, (HPG // halves) * (part + 1) - 1, s)
                        sync.wait_ge(sem_cpv, k + 1)
                        sync.wait_ge(sem_cps, k + 1)
                        sync.dma_start(
                            yv[s, :, g * GROUP + part * hw : g * GROUP + (part + 1) * hw],
                            st[:, slot * GROUP + part * hw : slot * GROUP + (part + 1) * hw],
                        ).then_inc(sem_out, 16)
                        n_dma += 1
            sync.wait_ge(sem_out, 16 * n_dma)

        @block.tensor
        def _(tensor):
            tensor.wait_ge(sem_in, 16)  # weights
            for g in range(NG):
                for h in range(HPG):
                    # wait for the input chunk covering this subtile
                    last_col = g * GROUP + (h + 1) * PS_COLS - 1
                    tensor.wait_ge(sem_in, 16 * (last_col // IN_CHUNK + 2))
                    for s in range(2):
                        k = tile_idx(g, h, s)
                        slot = k % N_PS_SLOTS
                        if k >= N_PS_SLOTS:  # WAR: copies of tile k-4 done
                            tensor.wait_ge(sem_cpv, k - N_PS_SLOTS + 1)
                            tensor.wait_ge(sem_cps, k - N_PS_SLOTS + 1)
                    mms = {}
                    for j in range(2):  # interleave strips so MM pairs overlap
                        for s in range(2):
                            k = tile_idx(g, h, s)
                            slot = k % N_PS_SLOTS
                            col = g * GROUP + h * PS_COLS + j * MM_N
                            mms[s] = nc.tensor.matmul(
                                ps[
                                    :,
                                    slot * PS_COLS
                                    + j * MM_N : slot * PS_COLS
                                    + (j + 1) * MM_N,
                                ],
                                wt[s * 64 : (s + 1) * 64, :],
                                xbuf[s * 64 : (s + 1) * 64, col : col + MM_N],
                                start=True,
                                stop=True,
                            )
                    # strip-s tiles complete in s order on the strict-FIFO PE
                    mms[0].then_inc(sem_mm, 1)
                    mms[1].then_inc(sem_mm, 1)

        def copier(engine, lo, hi, sem_mine):
            for g in range(NG):
                for h in range(HPG):
                    for s in range(2):
                        k = tile_idx(g, h, s)
                        slot = k % N_PS_SLOTS
                        m = 2 * g + s
                        st_slot = m % N_ST_SLOTS
                        engine.wait_ge(sem_mm, k + 1)
                        if h == 0 and m >= N_ST_SLOTS:
                            engine.wait_ge(sem_out, 16 * (m - N_ST_SLOTS + 1))
                        dst = st[
                            :,
                            st_slot * GROUP
                            + h * PS_COLS
                            + lo : st_slot * GROUP
                            + h * PS_COLS
                            + hi,
                        ]
                        src = ps[:, slot * PS_COLS + lo : slot * PS_COLS + hi]
                        if engine is nc.vector:
                            nc.vector.tensor_copy(dst, src).then_inc(sem_mine, 1)
                        else:
                            nc.scalar.copy(dst, src).then_inc(sem_mine, 1)

        @block.vector
        def _(vector):
            copier(nc.vector, 0, DVE_COLS, sem_cpv)

        @block.scalar
        def _(scalar):
            scalar.dma_start(wt[:], w[:]).then_inc(sem_in, 16)
            for i in range(n_in):
                scalar.dma_start(
                    xbuf[:, i * IN_CHUNK : (i + 1) * IN_CHUNK],
                    xv[:, i * IN_CHUNK : (i + 1) * IN_CHUNK],
                ).then_inc(sem_in, 16)
            copier(nc.scalar, DVE_COLS, PS_COLS, sem_cps)

    nc.compile()
    _CACHE["nc"] = nc
    return nc


def _execute(x, band_scale, trace=False, tmpdir=None):
    from concourse.bass_utils import run_bass_kernel_spmd

    x = np.ascontiguousarray(np.asarray(x, dtype=np.float32))
    band_scale = np.asarray(band_scale, dtype=np.float32)
    assert x.shape == (B, C, H, W), x.shape
    nc = _build_raw()
    wmat = _CACHE.get("w")
    if wmat is None:
        wmat = _CACHE["w"] = _band_weights()
    in_maps = [
        {"xv": _pack(x[c * B_LOC : (c + 1) * B_LOC]), "w": wmat}
        for c in range(N_CORES)
    ]
    res = run_bass_kernel_spmd(
        nc, in_maps, list(range(N_CORES)), trace=trace, tmpdir=tmpdir
    )
    low_parts, mid_parts = [], []
    for c in range(N_CORES):
        ya, yb = res.results[c]["yv"]
        low_parts.append(_unpack(np.concatenate([ya[:64], yb[:64]], axis=0)))
        mid_parts.append(_unpack(np.concatenate([ya[64:], yb[64:]], axis=0)))
    low_u = np.concatenate(low_parts, axis=0)
    mid_u = np.concatenate(mid_parts, axis=0)
    high_u = x - low_u - mid_u
    s0, s1, s2 = (float(band_scale[i]) for i in range(3))
    low = low_u if s0 == 1.0 else low_u * np.float32(s0)
    mid = mid_u if s1 == 1.0 else mid_u * np.float32(s1)
    high = high_u if s2 == 1.0 else high_u * np.float32(s2)
    return (low, mid, high), res


def kernel(x, band_scale):
    out, _ = _execute(x, band_scale)
    return out


# revision 17
# speedup vs baseline: 1.1112x; 1.1112x over previous
"""DCT band-decomposition kernel for Trainium2 (8 NeuronCores, SPMD).

Math: for each 8x8 block X of the input image, the reference computes
C = D @ X @ D^T, then per band b: out_b = D^T @ (M_b * C) @ D * scale_b.
This is linear in the 64 block elements, so each band is one fixed 64x64
matrix A_b = (D^T (x) D^T) diag(vec(M_b)) (D (x) D) applied to the
block-vectorized input.  Masks partition the spectrum, so
A_low + A_mid + A_high = I  =>  high_u = x - low_u - mid_u (done on host).

Device work per core (batch-sharded, 2 images of [3,512,512] per core):
  - input xv [128, 12288] f32: partitions 0-63 hold even blocks
    (vectorized 8x8), partitions 64-127 odd blocks; free = block pairs.
  - Two row-tiled matmuls per 512-column chunk run concurrently in
    disjoint PE row strips (K=64 each, auto tile_position from base
    partition 0 / 64).  lhsT = [A_low^T | A_mid^T] (M=128 = both bands).
  - PSUM -> DVE/ACT copies -> SBUF staging -> 2 MiB HWDGE DMA out.
  - Input streamed in 1 MiB SWDGE chunks so in/out DMA interleave.
Host folds band_scale (ones in practice) and restores natural layout.
"""

import numpy as np

N_CORES = 8
B, C, H, W = 16, 3, 512, 512
NB = 8  # DCT block size
B_LOC = B // N_CORES  # 2 images per core
NBH, NBW = H // NB, W // NB  # 64 x 64 blocks
FREE = B_LOC * C * NBH * (NBW // 2)  # 12288 block-pair columns per core
IN_CHUNK = 2048  # input DMA chunk (1 MiB)
GROUP = 2048  # columns per output DMA (1 MiB)
NG = FREE // GROUP  # 6
PS_COLS = 1024  # PSUM tile = 2 banks
MM_N = 512  # matmul free dim (one PSUM bank of f32)
DVE_COLS = 640  # per-PSUM-tile copy split: DVE 640 cols, ACT 384


def _dct_matrix(n):
    m = np.zeros((n, n), dtype=np.float64)
    for k in range(n):
        for t in range(n):
            if k == 0:
                m[k, t] = np.sqrt(1.0 / n)
            else:
                m[k, t] = np.sqrt(2.0 / n) * np.cos(np.pi * k * (2 * t + 1) / (2 * n))
    return m


def _zigzag(n):
    idxs = np.zeros((n, n), dtype=np.int64)
    idx = 0
    for s in range(2 * n - 1):
        if s % 2 == 0:
            rng = range(min(s, n - 1), max(0, s - n + 1) - 1, -1)
        else:
            rng = range(max(0, s - n + 1), min(s, n - 1) + 1)
        for i in rng:
            j = s - i
            if 0 <= i < n and 0 <= j < n:
                idxs[i, j] = idx
                idx += 1
    return idxs


def _band_weights():
    """lhsT [128, 128]: rows 0-63 / 64-127 both hold [A_low^T | A_mid^T]."""
    D = _dct_matrix(NB)
    zz = _zigzag(NB)
    total = NB * NB
    lo_t, hi_t = total // 3, 2 * total // 3
    low = (zz < lo_t).astype(np.float64)
    mid = ((zz >= lo_t) & (zz < hi_t)).astype(np.float64)
    K64 = np.kron(D, D)  # vec_row(D X D^T) = K64 @ vec_row(X)
    blocks = []
    for mask in (low, mid):
        A = K64.T @ (mask.reshape(-1)[:, None] * K64)  # 64x64 band operator
        blocks.append(A.T)  # out = lhsT.T @ rhs
    wA = np.concatenate(blocks, axis=1)  # [64, 128]
    w = np.concatenate([wA, wA], axis=0)  # [128, 128]
    return np.ascontiguousarray(w.astype(np.float32))


def _pack(xc):
    """[2,3,512,512] -> [128, 12288] block-pair-vectorized layout."""
    # (b, ch, bh, r, bw2, par, c) <- H = bh*8 + r ; W = bw2*16 + par*8 + c
    v = xc.reshape(B_LOC, C, NBH, NB, NBW // 2, 2, NB)
    v = v.transpose(5, 3, 6, 0, 1, 2, 4)  # (par, r, c, b, ch, bh, bw2)
    return np.ascontiguousarray(v.reshape(128, FREE))


def _unpack(yv):
    """[128, 12288] -> [2,3,512,512] (inverse of _pack)."""
    v = yv.reshape(2, NB, NB, B_LOC, C, NBH, NBW // 2)
    v = v.transpose(3, 4, 5, 1, 6, 0, 2)  # (b, ch, bh, r, bw2, par, c)
    return v.reshape(B_LOC, C, H, W)


IN_CHUNK = 2048  # 1 MiB input chunks
GROUP = 4096  # output DMA cols per stream (2 MiB)
NG = FREE // GROUP  # 3
PS_COLS = 1024
HPG = GROUP // PS_COLS  # 4 psum subtiles per output group
MM_N = 512
DVE_COLS = 640
N_PS_SLOTS = 4  # [128,1024] = 2 banks each -> all 8 PSUM banks
N_ST_SLOTS = 4  # stage ring [128, 2048] each

_CACHE = {}


def _build_raw():
    if "nc" in _CACHE:
        return _CACHE["nc"]
    from concourse import bacc, mybir

    f32 = mybir.dt.float32
    nc = bacc.Bacc("TRN2", target_bir_lowering=False, debug=False, num_devices=N_CORES)
    xv = nc.dram_tensor("xv", [128, FREE], f32, kind="ExternalInput").ap()
    w = nc.dram_tensor("w", [128, 128], f32, kind="ExternalInput").ap()
    yv = nc.dram_tensor("yv", [2, 128, FREE], f32, kind="ExternalOutput").ap()

    xbuf = nc.alloc_sbuf_tensor("xbuf", [128, FREE], f32).ap()
    wt = nc.alloc_sbuf_tensor("wt", [128, 128], f32).ap()
    st = nc.alloc_sbuf_tensor("st", [128, N_ST_SLOTS * GROUP], f32).ap()
    ps = nc.alloc_psum_tensor("ps", [128, N_PS_SLOTS * PS_COLS], f32).ap()

    n_in = FREE // IN_CHUNK  # 6
    n_out = NG * 2  # 12 output DMAs: (g, s)

    def tile_idx(g, h, s):
        return (g * HPG + h) * 2 + s

    with (
        # GpSimd issues no DMAs in this kernel, so skip its dge_drain in
        # the block-exit barrier (sem-only barrier instead).
        nc.Block(no_gpsimd_drain=True) as block,
        nc.semaphore("sem_in") as sem_in,
        nc.semaphore("sem_mm") as sem_mm,
        nc.semaphore("sem_cpv") as sem_cpv,
        nc.semaphore("sem_cps") as sem_cps,
        nc.semaphore("sem_out") as sem_out,
    ):
        # Inputs ride the ACT HWDGE ring (issued at the head of the scalar
        # program), outputs ride the SP ring, so output bytes can start
        # moving while input chunks are still streaming.
        @block.sync
        def _(sync):
            n_dma = 0
            for g in range(NG):
                for s in range(2):
                    m = 2 * g + s
                    slot = m % N_ST_SLOTS
                    # Split the final group's transfers in half: the last
                    # DMA is a serial tail after the last copies, so the
                    # first half can overlap the second half's copies.
                    halves = 2 if g == NG - 1 else 1
                    hw = GROUP // halves
                    for part in range(halves):
                        k = tile_idx(g, (HPG // halves) * (part + 1) - 1, s)
                        sync.wait_ge(sem_cpv, k + 1)
                        sync.wait_ge(sem_cps, k + 1)
                        sync.dma_start(
                            yv[s, :, g * GROUP + part * hw : g * GROUP + (part + 1) * hw],
                            st[:, slot * GROUP + part * hw : slot * GROUP + (part + 1) * hw],
                        ).then_inc(sem_out, 16)
                        n_dma += 1
            sync.wait_ge(sem_out, 16 * n_dma)

        @block.tensor
        def _(tensor):
            tensor.wait_ge(sem_in, 16)  # weights
            for g in range(NG):
                for h in range(HPG):
                    # wait for the input chunk covering this subtile
                    last_col = g * GROUP + (h + 1) * PS_COLS - 1
                    tensor.wait_ge(sem_in, 16 * (last_col // IN_CHUNK + 2))
                    for s in range(2):
                        k = tile_idx(g, h, s)
                        slot = k % N_PS_SLOTS
                        if k >= N_PS_SLOTS:  # WAR: copies of tile k-4 done
                            tensor.wait_ge(sem_cpv, k - N_PS_SLOTS + 1)
                            tensor.wait_ge(sem_cps, k - N_PS_SLOTS + 1)
                    mms = {}
                    for j in range(2):  # interleave strips so MM pairs overlap
                        for s in range(2):
                            k = tile_idx(g, h, s)
                            slot = k % N_PS_SLOTS
                            col = g * GROUP + h * PS_COLS + j * MM_N
                            mms[s] = nc.tensor.matmul(
                                ps[
                                    :,
                                    slot * PS_COLS
                                    + j * MM_N : slot * PS_COLS
                                    + (j + 1) * MM_N,
                                ],
                                wt[s * 64 : (s + 1) * 64, :],
                                xbuf[s * 64 : (s + 1) * 64, col : col + MM_N],
                                start=True,
                                stop=True,
                            )
                    # strip-s tiles complete in s order on the strict-FIFO PE
                    mms[0].then_inc(sem_mm, 1)
                    mms[1].then_inc(sem_mm, 1)

        def copier(engine, lo, hi, sem_mine):
            for g in range(NG):
                for h in range(HPG):
                    for s in range(2):
                        k = tile_idx(g, h, s)
                        slot = k % N_PS_SLOTS
                        m = 2 * g + s
                        st_slot = m % N_ST_SLOTS
                        engine.wait_ge(sem_mm, k + 1)
                        if h == 0 and m >= N_ST_SLOTS:
                            engine.wait_ge(sem_out, 16 * (m - N_ST_SLOTS + 1))
                        dst = st[
                            :,
                            st_slot * GROUP
                            + h * PS_COLS
                            + lo : st_slot * GROUP
                            + h * PS_COLS
                            + hi,
                        ]
                        src = ps[:, slot * PS_COLS + lo : slot * PS_COLS + hi]
                        if engine is nc.vector:
                            nc.vector.tensor_copy(dst, src).then_inc(sem_mine, 1)
                        else:
                            nc.scalar.copy(dst, src).then_inc(sem_mine, 1)

        @block.vector
        def _(vector):
            copier(nc.vector, 0, DVE_COLS, sem_cpv)

        @block.scalar
        def _(scalar):
            scalar.dma_start(wt[:], w[:]).then_inc(sem_in, 16)
            for i in range(n_in):
                scalar.dma_start(
                    xbuf[:, i * IN_CHUNK : (i + 1) * IN_CHUNK],
                    xv[:, i * IN_CHUNK : (i + 1) * IN_CHUNK],
                ).then_inc(sem_in, 16)
            copier(nc.scalar, DVE_COLS, PS_COLS, sem_cps)

    nc.compile()
    _CACHE["nc"] = nc
    return nc


def _execute(x, band_scale, trace=False, tmpdir=None):
    from concourse.bass_utils import run_bass_kernel_spmd

    x = np.ascontiguousarray(np.asarray(x, dtype=np.float32))
    band_scale = np.asarray(band_scale, dtype=np.float32)
    assert x.shape == (B, C, H, W), x.shape
    nc = _build_raw()
    wmat = _CACHE.get("w")
    if wmat is None:
        wmat = _CACHE["w"] = _band_weights()
    in_maps = [
        {"xv": _pack(x[c * B_LOC : (c + 1) * B_LOC]), "w": wmat}
        for c in range(N_CORES)
    ]
    res = run_bass_kernel_spmd(
        nc, in_maps, list(range(N_CORES)), trace=trace, tmpdir=tmpdir
    )
    low_parts, mid_parts = [], []
    for c in range(N_CORES):
        ya, yb = res.results[c]["yv"]
        low_parts.append(_unpack(np.concatenate([ya[:64], yb[:64]], axis=0)))
        mid_parts.append(_unpack(np.concatenate([ya[64:], yb[64:]], axis=0)))
    low_u = np.concatenate(low_parts, axis=0)
    mid_u = np.concatenate(mid_parts, axis=0)
    high_u = x - low_u - mid_u
    s0, s1, s2 = (float(band_scale[i]) for i in range(3))
    low = low_u if s0 == 1.0 else low_u * np.float32(s0)
    mid = mid_u if s1 == 1.0 else mid_u * np.float32(s1)
    high = high_u if s2 == 1.0 else high_u * np.float32(s2)
    return (low, mid, high), res


def kernel(x, band_scale):
    out, _ = _execute(x, band_scale)
    return out
